# revision 46
# baseline (speedup 1.0000x reference)
"""Trainium2 Bass kernel for nn_Attention_Weighted_Context_Generation.

Computes ctx = A @ F where
  A = weights.reshape(9216, 9216)              (row i = output location)
  F = cnn_feature.reshape(256, 9216).T          [9216, 256]
and returns ctx.reshape(9216, 1, 1, 256) float32.

Sharding: rows of A (the HW/location dim) split across 8 NeuronCores,
1152 rows each; F replicated (per the sharding hint). Each core's shard
is packed host-side in bfloat16 (the kernel is memory-bound; bf16
halves HBM traffic to ~26 MB/core at 2.3e-3 rel err vs the 2e-2 gate)
as a [4608, 2816] array whose 128-partition line p of super-tile j
holds [At | F] rows for k = j*256+p and j*256+128+p side by side
(5632-byte DMA descriptor lines, two 128-deep k-tiles per dma_start).

Device loop: stream 36 super-tiles through an 8-slot SBUF ring fed by
two HWDGE queues (sync=even, scalar=odd; a single queue serializes
dma_starts against their own transfers). Matmuls run F-STATIONARY:
per k-tile the two 128-col halves of the F tile are the stationary
operand and the packed A^T line streams as the moving operand in
512|512|128-col m-slices, accumulating ctx^T [256c, 1152m] in six
bank-aligned PSUM chains. This cuts weight loads 648 -> 432 and makes
the PE stream run at the back-to-back ideal (N cycles @ 2.4 GHz;
A-stationary paid +15-25% per matmul because a 97 ns LDWEIGHTS cannot
hide under a 109 ns 256-col matmul). Dummy warm-up matmuls bridge the
HAM clock-gate ramp from block start to the first tile's arrival.
PSUM is evacuated per c-half (DVE / ACT) in 576-col chunks with the
bf16 store DMAs overlapping the next chunk's evac; the host transposes
ctx^T back and casts to f32.

Measured on trn2 (8 cores): ~93-99 us/core NEFF exec (from 172 us for
the fp32r A-stationary baseline); PE-dense at ~72 us with DMA feeding
at ~360 GB/s/core.
"""

import numpy as np

import concourse.bass as bass
from concourse import mybir
from concourse.bass_utils import run_bass_kernel_spmd

N_CORES = 8
HW = 9216              # number of locations = 96*96
C = 256                # channels
M_PER = HW // N_CORES  # 1152 output rows per core
KT = HW // 128         # 72 contraction tiles
MT = M_PER // 128      # 9 output row-tiles per core
W_COLS = M_PER + C     # 1408 packed columns per k-row
KPD = 2                # k-tiles packed per DMA line (5632B descriptor
                       # lines; KPD=3 was tried and the per-dma_start
                       # queue gap did not shrink, it just delayed the
                       # first tile)
KT2 = KT // KPD        # 36 DMA super-tiles
L_COLS = KPD * W_COLS  # 2816 packed columns per super-tile line
NBUF = 8               # SBUF ring depth for streamed super-tiles; deep so
                       # the DMA stream decouples from PE hiccups (PE and
                       # DMA per-super-tile times are nearly equal)
NSEM = 12              # rotation depth for DMA-completion semaphores
                       # (> NBUF in-flight DMAs)
# F-stationary matmul layout: ctx^T[c, m] = sum_k F[k, c] * At[k, m].
# F k-tiles are the stationary operand (two 128-col c-halves), the packed
# A^T lines stream as the moving operand in three m-slices per c-half.
# PSUM is bank-aligned: each c-half chain owns 3 banks (512|512|128 cols,
# padded to 1536), so every slice is its bank's sole occupant and can
# start=True on the first k-tile (start clears the WHOLE bank).
CH_STRIDE = 1536       # PSUM cols reserved per c-half chain (3 banks)
M_SLICES = ((0, 512), (512, 512), (1024, 128))

# PE compute dtype. bfloat16 halves HBM traffic vs fp32/f32r (the kernel
# is memory-bound: 26MB vs 52MB per core) at the same PE rate (1 output
# row/cycle, same as f32r at N>=256). The cast happens HOST-side in
# prep_inputs — the atf array in HBM is already bf16, so loads stay on
# the fast HWDGE path (no SWDGE cast-in-DMA). Accuracy: inputs quantized
# to bf16, accumulation in fp32 PSUM -> ~2.6e-3 rel err vs the fp32
# reference (gate is 2e-2). float32r (1.4e-4, ~2x slower) remains valid
# and can be swapped back in here.
COMPUTE_DT = mybir.dt.bfloat16


def cast_loads_global() -> bool:
    """Host packs atf in COMPUTE_DT, so DMAs never cast (HWDGE path)."""
    return False


def build_bass():
    nc = bass.Bass("TRN2", target_bir_lowering=False, debug=False,
                   num_devices=N_CORES)
    # atf is packed host-side in COMPUTE_DT, so the load DMA is a plain
    # (non-casting) HWDGE copy. Row p of super-tile jt2 holds k-rows
    # jt2*256+p and jt2*256+128+p side by side (KPD k-tiles per line).
    atf = nc.dram_tensor("atf", [KT2 * 128, L_COLS], COMPUTE_DT,
                         kind="ExternalInput").ap()
    # Output is ctx^T for this core's row block: [256 c, 1152 m] bf16.
    # The host transposes back (free) and casts to f32.
    out = nc.dram_tensor("out", [C, M_PER], mybir.dt.bfloat16,
                         kind="ExternalOutput").ap()

    from contextlib import ExitStack
    with (
        ExitStack() as stack,
        nc.sbuf_tensor("kbufs", [128, NBUF * L_COLS], COMPUTE_DT) as kbufs,
        # Output staged in bf16: halves PSUM-evac time (2x DVE rate for
        # 16-bit) and the store DMA. Output quantization adds ~2e-3 rel
        # err in quadrature -- way under the 2e-2 gate. The host casts
        # back to f32.
        nc.sbuf_tensor("out_sb", [128, 2 * M_PER], mybir.dt.bfloat16) as out_sb,
        nc.psum_tensor("acc", [128, 2 * CH_STRIDE], mybir.dt.float32) as acc,
        # scratch bank for PE warm-up matmuls (never read)
        nc.psum_tensor("warm", [128, 512], mybir.dt.float32) as warm,
        nc.semaphore("mm_sem") as mm_sem,
        nc.semaphore("first_sem") as first_sem,
        nc.semaphore("bank_sem") as bank_sem,
        nc.semaphore("dve_done") as dve_done,
        nc.semaphore("act_done") as act_done,
        nc.semaphore("out_sem") as out_sem,
        # No gpsimd DMAs are issued on the f32r path, so skip GpSimd's
        # dge_drain at Block exit and use the sem-only exit barrier
        # (shaves part of the ~8us kernel-tail butterfly).
        nc.Block(no_gpsimd_drain=not cast_loads_global()) as block,
    ):
        # DMA-completion sems must rotate: a dma_start completes as 16
        # independent per-SDMA-engine increments, and increments of
        # consecutive DMAs interleave across engines. With a single shared
        # sem, "sem >= 16*(jt+1)" does NOT imply DMA jt's data landed
        # (NTFF traces showed the sem leading the last data packet by
        # ~850ns -> stale-tile matmuls, nondeterministic results).
        # Per-engine descriptor FIFO makes a rotation of NSEM sems safe
        # against up to NSEM-1 DMAs of cross-engine skew.
        dma_sems = [stack.enter_context(nc.semaphore(f"dma_sem{i}"))
                    for i in range(NSEM)]

        # atf is packed host-side in COMPUTE_DT, so loads never cast and
        # always take the faster HWDGE (sync) path.
        cast_loads = cast_loads_global()

        def emit_loads(eng, phase, nphase):
            # Loads are split across the two available HWDGE queues
            # (sync: even jt, scalar: odd jt; DVE cannot issue DMAs) so
            # one queue's descriptor-generation bubble is covered by the
            # other queue's transfers. Super-tile 0 is loaded as two
            # half-lines (one per queue) so the PE's first sub-k-tile is
            # ready in ~half the transfer time -- the only PE idle in the
            # whole stream is waiting for this first tile.
            if phase == 0:
                eng.dma_start(
                    out=kbufs[:, :W_COLS],
                    in_=atf[0:128, :W_COLS],
                ).then_inc(dma_sems[0], 16)
            else:
                eng.dma_start(
                    out=kbufs[:, W_COLS:L_COLS],
                    in_=atf[0:128, W_COLS:],
                ).then_inc(first_sem, 16)
            for jt in range(nphase - phase, KT2, nphase):
                if jt >= NBUF:
                    # ring slot reused: wait until its matmuls retired
                    eng.wait_ge(mm_sem, jt - NBUF + 1)
                b = jt % NBUF
                eng.dma_start(
                    out=kbufs[:, b * L_COLS:(b + 1) * L_COLS],
                    in_=atf[jt * 128:(jt + 1) * 128, :],
                ).then_inc(dma_sems[jt % NSEM], 16)

        @block.sync
        def _(sync):
            emit_loads(sync, 0, 2)
            # store c-half 0 (out rows 0:128) bank-chunk by bank-chunk as
            # DVE evacuates each one
            for h, (m0, n) in enumerate(M_SLICES):
                sync.wait_ge(dve_done, h + 1)
                sync.dma_start(
                    out=out[:128, m0:m0 + n],
                    in_=out_sb[:, m0:m0 + n],
                ).then_inc(out_sem, 16)
            sync.wait_ge(out_sem, 96)

        @block.tensor
        def _(tensor):
            # PE warm-up: HAM clock-gates an idle PE, and the first ~3.5us
            # of matmuls after an idle gap run at a reduced p-state. Dummy
            # matmuls on garbage SBUF (result never read) start the ramp at
            # block begin; the later ones gate on the first DMA's partial
            # per-engine sem increments so the PE stays busy through the
            # transfer window instead of re-throttling.
            # Operands read out_sb, which no DMA touches until the tail --
            # reading kbufs here raced the first load's landing data.
            for v in (0, 0, 0, 4, 8, 12):
                if v:
                    tensor.wait_ge(dma_sems[0], v)
                tensor.matmul(warm[:, :512], out_sb[:, 1152:1280],
                              out_sb[:, 576:1088], start=True, stop=True)
            for jt in range(KT2):
                tensor.wait_ge(dma_sems[jt % NSEM], 16 * (jt // NSEM + 1))
                b = jt % NBUF
                inst = None
                for s in range(KPD):
                    if jt == 0 and s == 1:
                        # second half-line of the split first load
                        tensor.wait_ge(first_sem, 16)
                    buf = kbufs[:, b * L_COLS + s * W_COLS:
                                b * L_COLS + (s + 1) * W_COLS]
                    at = buf[:, :M_PER]            # [128 k, 1152 m] moving
                    f_tile = buf[:, M_PER:W_COLS]  # [128 k, 256 c] stationary
                    last = (jt == KT2 - 1 and s == KPD - 1)
                    # c-half 1 first: its evac path (ACT copy + store, one
                    # engine) is the longer tail, so release its banks
                    # first on the final group. Each m-slice IS one PSUM
                    # bank, so per-slice bank_sem increments on the final
                    # group let the evac engines pipeline bank-by-bank
                    # behind the PE (disjoint banks, collision-safe).
                    for ch in (1, 0):
                        lhsT = f_tile[:, ch * 128:(ch + 1) * 128]
                        for m0, n in M_SLICES:
                            inst = tensor.matmul(
                                acc[:, ch * CH_STRIDE + m0:
                                    ch * CH_STRIDE + m0 + n],
                                lhsT,
                                at[:, m0:m0 + n],
                                start=(jt == 0 and s == 0),
                                stop=last,
                            )
                            if last:
                                inst.then_inc(bank_sem, 1)
                if jt < KT2 - 1:
                    inst.then_inc(mm_sem, 1)

        @block.vector
        def _(vector):
            # c-half 0 (psum banks 0-2) -> out_sb cols 0:1152 (bf16 cast),
            # one bank behind the PE's final slices (bank_sem 4,5,6)
            for h, (m0, n) in enumerate(M_SLICES):
                vector.wait_ge(bank_sem, 4 + h)
                vector.tensor_copy(
                    out_sb[:, m0:m0 + n],
                    acc[:, m0:m0 + n]).then_inc(dve_done, 1)

        @block.scalar
        def _(scalar):
            # Warm the ACT table early: the first ACTIVATE after boot pays a
            # ~1.4us cold-table cost; a 1-element copy during the stream
            # moves that off the critical tail (the garbage written to
            # out_sb[0,0] is overwritten by the real evacuation below).
            scalar.copy(out_sb[:1, :1], out_sb[:1, :1])
            emit_loads(scalar, 1, 2)
            # c-half 1 (psum banks 3-5); store from ACT's own HWDGE ring,
            # concurrent with sync's store of the DVE half, pipelined
            # bank-by-bank behind the PE's final slices (bank_sem 1,2,3).
            for h, (m0, n) in enumerate(M_SLICES):
                scalar.wait_ge(bank_sem, h + 1)
                scalar.copy(
                    out_sb[:, M_PER + m0:M_PER + m0 + n],
                    acc[:, CH_STRIDE + m0:CH_STRIDE + m0 + n]
                ).then_inc(act_done, 1)
                scalar.wait_ge(act_done, h + 1)
                scalar.dma_start(
                    out=out[128:, m0:m0 + n],
                    in_=out_sb[:, M_PER + m0:M_PER + m0 + n],
                ).then_inc(out_sem, 16)

    return nc


def prep_inputs(weights: np.ndarray, cnn_feature: np.ndarray):
    """Pack per-core [4608, 2816] COMPUTE_DT arrays.

    Logical layout is [HW, W_COLS] rows of [A_shard^T | F]; KPD k-tiles
    are then packed side by side per 128-partition line so each DMA
    descriptor line is KPD*W_COLS elements.
    """
    np_dt = mybir.dt.np(COMPUTE_DT)
    A = (np.asarray(weights, dtype=np.float32).reshape(HW, HW)
         .astype(np_dt))
    F = (np.asarray(cnn_feature, dtype=np.float32).reshape(C, HW).T
         .astype(np_dt))  # [HW, C]
    in_maps = []
    for i in range(N_CORES):
        at = A[i * M_PER:(i + 1) * M_PER, :].T  # [HW, M_PER] view
        atf = np.concatenate([at, F], axis=1)   # [HW, 1408] contiguous
        atf = (atf.reshape(KT2, KPD, 128, W_COLS)
               .transpose(0, 2, 1, 3)
               .reshape(KT2 * 128, L_COLS))
        in_maps.append({"atf": np.ascontiguousarray(atf)})
    return in_maps


def kernel(weights: np.ndarray, cnn_feature: np.ndarray) -> np.ndarray:
    in_maps = prep_inputs(weights, cnn_feature)
    nc = build_bass()
    res = run_bass_kernel_spmd(nc, in_maps, list(range(N_CORES)))
    # each core returns ctx^T [256, 1152] bf16; transpose + cast back
    ctx = np.concatenate(
        [np.asarray(res.results[i]["out"], dtype=np.float32).T
         for i in range(N_CORES)], axis=0)
    return np.ascontiguousarray(ctx.reshape(HW, 1, 1, C))



# revision 47
# speedup vs baseline: 1.0502x; 1.0502x over previous
"""Trainium2 Bass kernel for nn_Attention_Weighted_Context_Generation.

Computes ctx = A @ F where
  A = weights.reshape(9216, 9216)              (row i = output location)
  F = cnn_feature.reshape(256, 9216).T          [9216, 256]
and returns ctx.reshape(9216, 1, 1, 256) float32.

Sharding: rows of A (the HW/location dim) split across 8 NeuronCores,
1152 rows each; F replicated (per the sharding hint). Each core's shard
is packed host-side in bfloat16 (the kernel is memory-bound; bf16
halves HBM traffic to ~26 MB/core at 2.3e-3 rel err vs the 2e-2 gate)
as a [4608, 2816] array whose 128-partition line p of super-tile j
holds [At | F] rows for k = j*256+p and j*256+128+p side by side
(5632-byte DMA descriptor lines, two 128-deep k-tiles per dma_start).

Device loop: stream 36 super-tiles through an 8-slot SBUF ring fed by
two HWDGE queues (sync=even, scalar=odd; a single queue serializes
dma_starts against their own transfers). Matmuls run F-STATIONARY:
per k-tile the two 128-col halves of the F tile are the stationary
operand and the packed A^T line streams as the moving operand in
512|512|128-col m-slices, accumulating ctx^T [256c, 1152m] in six
bank-aligned PSUM chains. This cuts weight loads 648 -> 432 and makes
the PE stream run at the back-to-back ideal (N cycles @ 2.4 GHz;
A-stationary paid +15-25% per matmul because a 97 ns LDWEIGHTS cannot
hide under a 109 ns 256-col matmul). Dummy warm-up matmuls bridge the
HAM clock-gate ramp from block start to the first tile's arrival.
PSUM is evacuated per c-half (DVE / ACT) in 576-col chunks with the
bf16 store DMAs overlapping the next chunk's evac; the host transposes
ctx^T back and casts to f32.

Measured on trn2 (8 cores): ~93-99 us/core NEFF exec (from 172 us for
the fp32r A-stationary baseline); PE-dense at ~72 us with DMA feeding
at ~360 GB/s/core.
"""

import numpy as np

import concourse.bass as bass
from concourse import mybir
from concourse.bass_utils import run_bass_kernel_spmd

N_CORES = 8
HW = 9216              # number of locations = 96*96
C = 256                # channels
M_PER = HW // N_CORES  # 1152 output rows per core
KT = HW // 128         # 72 contraction tiles
MT = M_PER // 128      # 9 output row-tiles per core
W_COLS = M_PER + C     # 1408 packed columns per k-row
KPD = 2                # k-tiles packed per DMA line (5632B descriptor
                       # lines; KPD=3 was tried and the per-dma_start
                       # queue gap did not shrink, it just delayed the
                       # first tile)
KT2 = KT // KPD        # 36 DMA super-tiles
L_COLS = KPD * W_COLS  # 2816 packed columns per super-tile line
NBUF = 8               # SBUF ring depth for streamed super-tiles; deep so
                       # the DMA stream decouples from PE hiccups (PE and
                       # DMA per-super-tile times are nearly equal)
NSEM = 12              # rotation depth for DMA-completion semaphores
                       # (> NBUF in-flight DMAs)
# F-stationary matmul layout: ctx^T[c, m] = sum_k F[k, c] * At[k, m].
# F k-tiles are the stationary operand (two 128-col c-halves), the packed
# A^T lines stream as the moving operand in three m-slices per c-half.
# PSUM is bank-aligned: each c-half chain owns 3 banks (512|512|128 cols,
# padded to 1536), so every slice is its bank's sole occupant and can
# start=True on the first k-tile (start clears the WHOLE bank).
CH_STRIDE = 1536       # PSUM cols reserved per c-half chain (3 banks)
M_SLICES = ((0, 512), (512, 512), (1024, 128))

# PE compute dtype. bfloat16 halves HBM traffic vs fp32/f32r (the kernel
# is memory-bound: 26MB vs 52MB per core) at the same PE rate (1 output
# row/cycle, same as f32r at N>=256). The cast happens HOST-side in
# prep_inputs — the atf array in HBM is already bf16, so loads stay on
# the fast HWDGE path (no SWDGE cast-in-DMA). Accuracy: inputs quantized
# to bf16, accumulation in fp32 PSUM -> ~2.6e-3 rel err vs the fp32
# reference (gate is 2e-2). float32r (1.4e-4, ~2x slower) remains valid
# and can be swapped back in here.
COMPUTE_DT = mybir.dt.bfloat16


def cast_loads_global() -> bool:
    """Host packs atf in COMPUTE_DT, so DMAs never cast (HWDGE path)."""
    return False


def build_bass():
    nc = bass.Bass("TRN2", target_bir_lowering=False, debug=False,
                   num_devices=N_CORES)
    # atf is packed host-side in COMPUTE_DT, so the load DMA is a plain
    # (non-casting) HWDGE copy. Row p of super-tile jt2 holds k-rows
    # jt2*256+p and jt2*256+128+p side by side (KPD k-tiles per line).
    atf = nc.dram_tensor("atf", [KT2 * 128, L_COLS], COMPUTE_DT,
                         kind="ExternalInput").ap()
    # Output is ctx^T for this core's row block: [256 c, 1152 m] bf16.
    # The host transposes back (free) and casts to f32.
    out = nc.dram_tensor("out", [C, M_PER], mybir.dt.bfloat16,
                         kind="ExternalOutput").ap()

    from contextlib import ExitStack
    with (
        ExitStack() as stack,
        nc.sbuf_tensor("kbufs", [128, NBUF * L_COLS], COMPUTE_DT) as kbufs,
        # Output staged in bf16: halves PSUM-evac time (2x DVE rate for
        # 16-bit) and the store DMA. Output quantization adds ~2e-3 rel
        # err in quadrature -- way under the 2e-2 gate. The host casts
        # back to f32.
        nc.sbuf_tensor("out_sb", [128, 2 * M_PER], mybir.dt.bfloat16) as out_sb,
        nc.psum_tensor("acc", [128, 2 * CH_STRIDE], mybir.dt.float32) as acc,
        # scratch bank for PE warm-up matmuls (never read)
        nc.psum_tensor("warm", [128, 512], mybir.dt.float32) as warm,
        nc.semaphore("mm_sem") as mm_sem,
        nc.semaphore("bank_sem") as bank_sem,
        nc.semaphore("dve_done") as dve_done,
        nc.semaphore("act_done") as act_done,
        nc.semaphore("out_sem") as out_sem,
        # No gpsimd DMAs are issued on the f32r path, so skip GpSimd's
        # dge_drain at Block exit and use the sem-only exit barrier
        # (shaves part of the ~8us kernel-tail butterfly).
        nc.Block(no_gpsimd_drain=not cast_loads_global()) as block,
    ):
        # DMA-completion sems must rotate: a dma_start completes as 16
        # independent per-SDMA-engine increments, and increments of
        # consecutive DMAs interleave across engines. With a single shared
        # sem, "sem >= 16*(jt+1)" does NOT imply DMA jt's data landed
        # (NTFF traces showed the sem leading the last data packet by
        # ~850ns -> stale-tile matmuls, nondeterministic results).
        # Per-engine descriptor FIFO makes a rotation of NSEM sems safe
        # against up to NSEM-1 DMAs of cross-engine skew.
        dma_sems = [stack.enter_context(nc.semaphore(f"dma_sem{i}"))
                    for i in range(NSEM)]

        # atf is packed host-side in COMPUTE_DT, so loads never cast and
        # always take the faster HWDGE (sync) path.
        cast_loads = cast_loads_global()

        def emit_loads(eng, phase, nphase):
            # Loads are split across the two available HWDGE queues
            # (sync: even jt, scalar: odd jt; DVE cannot issue DMAs) so
            # one queue's descriptor-generation bubble is covered by the
            # other queue's transfers.
            for jt in range(phase, KT2, nphase):
                if jt >= NBUF:
                    # ring slot reused: wait until its matmuls retired
                    eng.wait_ge(mm_sem, jt - NBUF + 1)
                b = jt % NBUF
                eng.dma_start(
                    out=kbufs[:, b * L_COLS:(b + 1) * L_COLS],
                    in_=atf[jt * 128:(jt + 1) * 128, :],
                ).then_inc(dma_sems[jt % NSEM], 16)

        @block.sync
        def _(sync):
            emit_loads(sync, 0, 2)
            # store c-half 0 (out rows 0:128) bank-chunk by bank-chunk as
            # DVE evacuates each one
            for h, (m0, n) in enumerate(M_SLICES):
                sync.wait_ge(dve_done, h + 1)
                sync.dma_start(
                    out=out[:128, m0:m0 + n],
                    in_=out_sb[:, m0:m0 + n],
                ).then_inc(out_sem, 16)
            sync.wait_ge(out_sem, 96)

        @block.tensor
        def _(tensor):
            # PE warm-up: HAM clock-gates an idle PE, and the first ~3.5us
            # of matmuls after an idle gap run at a reduced p-state. Dummy
            # matmuls on garbage SBUF (result never read) start the ramp at
            # block begin; the later ones gate on the first DMA's partial
            # per-engine sem increments so the PE stays busy through the
            # transfer window instead of re-throttling.
            # Operands read out_sb, which no DMA touches until the tail --
            # reading kbufs here raced the first load's landing data.
            for v in (0, 0, 0, 4, 8, 12):
                if v:
                    tensor.wait_ge(dma_sems[0], v)
                tensor.matmul(warm[:, :512], out_sb[:, 1152:1280],
                              out_sb[:, 576:1088], start=True, stop=True)
            for jt in range(KT2):
                tensor.wait_ge(dma_sems[jt % NSEM], 16 * (jt // NSEM + 1))
                b = jt % NBUF
                inst = None
                for s in range(KPD):
                    buf = kbufs[:, b * L_COLS + s * W_COLS:
                                b * L_COLS + (s + 1) * W_COLS]
                    at = buf[:, :M_PER]            # [128 k, 1152 m] moving
                    f_tile = buf[:, M_PER:W_COLS]  # [128 k, 256 c] stationary
                    last = (jt == KT2 - 1 and s == KPD - 1)
                    # c-half 1 first: its evac path (ACT copy + store, one
                    # engine) is the longer tail, so release its banks
                    # first on the final group. Each m-slice IS one PSUM
                    # bank, so per-slice bank_sem increments on the final
                    # group let the evac engines pipeline bank-by-bank
                    # behind the PE (disjoint banks, collision-safe).
                    for ch in (1, 0):
                        lhsT = f_tile[:, ch * 128:(ch + 1) * 128]
                        for m0, n in M_SLICES:
                            inst = tensor.matmul(
                                acc[:, ch * CH_STRIDE + m0:
                                    ch * CH_STRIDE + m0 + n],
                                lhsT,
                                at[:, m0:m0 + n],
                                start=(jt == 0 and s == 0),
                                stop=last,
                            )
                            if last:
                                inst.then_inc(bank_sem, 1)
                if jt < KT2 - 1:
                    inst.then_inc(mm_sem, 1)

        @block.vector
        def _(vector):
            # c-half 0 (psum banks 0-2) -> out_sb cols 0:1152 (bf16 cast),
            # one bank behind the PE's final slices (bank_sem 4,5,6)
            for h, (m0, n) in enumerate(M_SLICES):
                vector.wait_ge(bank_sem, 4 + h)
                vector.tensor_copy(
                    out_sb[:, m0:m0 + n],
                    acc[:, m0:m0 + n]).then_inc(dve_done, 1)

        @block.scalar
        def _(scalar):
            # Warm the ACT table early: the first ACTIVATE after boot pays a
            # ~1.4us cold-table cost; a 1-element copy during the stream
            # moves that off the critical tail (the garbage written to
            # out_sb[0,0] is overwritten by the real evacuation below).
            scalar.copy(out_sb[:1, :1], out_sb[:1, :1])
            emit_loads(scalar, 1, 2)
            # c-half 1 (psum banks 3-5); store from ACT's own HWDGE ring,
            # concurrent with sync's store of the DVE half, pipelined
            # bank-by-bank behind the PE's final slices (bank_sem 1,2,3).
            for h, (m0, n) in enumerate(M_SLICES):
                scalar.wait_ge(bank_sem, h + 1)
                scalar.copy(
                    out_sb[:, M_PER + m0:M_PER + m0 + n],
                    acc[:, CH_STRIDE + m0:CH_STRIDE + m0 + n]
                ).then_inc(act_done, 1)
                scalar.wait_ge(act_done, h + 1)
                scalar.dma_start(
                    out=out[128:, m0:m0 + n],
                    in_=out_sb[:, M_PER + m0:M_PER + m0 + n],
                ).then_inc(out_sem, 16)

    return nc


def prep_inputs(weights: np.ndarray, cnn_feature: np.ndarray):
    """Pack per-core [4608, 2816] COMPUTE_DT arrays.

    Logical layout is [HW, W_COLS] rows of [A_shard^T | F]; KPD k-tiles
    are then packed side by side per 128-partition line so each DMA
    descriptor line is KPD*W_COLS elements.
    """
    np_dt = mybir.dt.np(COMPUTE_DT)
    A = (np.asarray(weights, dtype=np.float32).reshape(HW, HW)
         .astype(np_dt))
    F = (np.asarray(cnn_feature, dtype=np.float32).reshape(C, HW).T
         .astype(np_dt))  # [HW, C]
    in_maps = []
    for i in range(N_CORES):
        at = A[i * M_PER:(i + 1) * M_PER, :].T  # [HW, M_PER] view
        atf = np.concatenate([at, F], axis=1)   # [HW, 1408] contiguous
        atf = (atf.reshape(KT2, KPD, 128, W_COLS)
               .transpose(0, 2, 1, 3)
               .reshape(KT2 * 128, L_COLS))
        in_maps.append({"atf": np.ascontiguousarray(atf)})
    return in_maps


def kernel(weights: np.ndarray, cnn_feature: np.ndarray) -> np.ndarray:
    in_maps = prep_inputs(weights, cnn_feature)
    nc = build_bass()
    res = run_bass_kernel_spmd(nc, in_maps, list(range(N_CORES)))
    # each core returns ctx^T [256, 1152] bf16; transpose + cast back
    ctx = np.concatenate(
        [np.asarray(res.results[i]["out"], dtype=np.float32).T
         for i in range(N_CORES)], axis=0)
    return np.ascontiguousarray(ctx.reshape(HW, 1, 1, C))

